# revision 16
# baseline (speedup 1.0000x reference)
"""8-layer GraphSAGE (SAGEConv mean-aggregate) forward, sharded across 8 TRN2 NeuronCores.

Strategy:
  - Nodes sharded contiguously across cores (6250/core). Within a core, nodes are
    permuted into blocks of 64 "slots" balanced by in-degree.
  - bf16 compute, fp32 PSUM accumulation.
  - Layers where dout <= din are "transform-first" (aggregate h@Wl); layers where
    dout > din are "scatter-first" (aggregate h, then matmul).
  - Per layer: the table (hl or h, node-major bf16) is AllGathered to every core's
    DRAM; per-edge rows are fetched with dma_gather (int16 idx; two passes for the
    lo/hi halves of the table row space); scatter-sum is done on the TensorEngine
    with host-precomputed one-hot matrices whose values are 1/deg(dst) (so PSUM
    directly accumulates the mean); the h@Wr transform and +bias are fused into the
    same PSUM accumulation group as extra matmuls.
"""

from contextlib import ExitStack

import numpy as np
import ml_dtypes

import concourse.bass as bass
import concourse.bacc as bacc
import concourse.mybir as mybir
import concourse.tile as tile
from concourse.bass_utils import run_bass_kernel_spmd

P = 128
BLK = 64  # dst slots per scatter block (one-hot M dim)

BF16 = mybir.dt.bfloat16
F32 = mybir.dt.float32
I16 = mybir.dt.int16

REAL_DIMS = [500, 256, 128, 128, 128, 128, 128, 256, 500]
REAL_NORM = [False, False, False, True, False, False, False, False]


def _ceil(a, b):
    return -(-a // b)


def _pad128(x):
    return _ceil(x, P) * P


class Cfg:
    def __init__(self, N, E, C, dims, norm, half=32768, gtc=8, grp=2):
        self.N, self.E, self.C = N, E, C
        self.dims, self.norm = dims, norm
        self.L = len(dims) - 1
        self.HALF = half
        self.GTC = gtc  # gather chunks per G tile
        self.GRP = grp  # node-chunks per streamed-transform group (layer 1)
        self.NPC = N // C
        self.NB = _ceil(self.NPC, BLK)  # blocks of 64 slots
        assert self.NB % 2 == 0, "need even block count for pairing"
        self.SLOTS = self.NB * BLK
        self.R = C * self.SLOTS  # table rows
        assert self.R - half <= 32768 and half <= 32768
        # per-layer mode: 'T' transform-first, 'S' scatter-first
        self.mode = ["T" if dims[k + 1] <= dims[k] else "S" for k in range(self.L)]
        # gather/table dim per layer
        self.tdim = [dims[k + 1] if self.mode[k] == "T" else dims[k] for k in range(self.L)]
        for d in self.tdim:
            assert d % P == 0, f"table dim {d} must be multiple of 128"


class Plan:
    """Host-side static schedule shared by all cores (padded to cross-core max)."""


def make_plan(cfg, edge_index):
    src = edge_index[0].astype(np.int64)
    dst = edge_index[1].astype(np.int64)
    N, C, NPC, NB, SLOTS = cfg.N, cfg.C, cfg.NPC, cfg.NB, cfg.SLOTS

    deg = np.bincount(dst, minlength=N).astype(np.int64)
    inv = (1.0 / np.maximum(deg, 1)).astype(np.float32)

    # --- balanced slot assignment per core ---
    slot_of = np.empty(N, np.int64)  # local slot within core
    for c in range(C):
        nodes = np.arange(c * NPC, (c + 1) * NPC)
        order = nodes[np.argsort(-deg[nodes], kind="stable")]
        loads = np.zeros(NB, np.int64)
        counts = np.zeros(NB, np.int64)
        for n in order:
            open_b = np.flatnonzero(counts < BLK)
            b = open_b[np.argmin(loads[open_b])]
            slot_of[n] = b * BLK + counts[b]
            loads[b] += deg[n]
            counts[b] += 1
    row_of = (np.arange(N) // NPC) * SLOTS + slot_of  # global table row

    ecore = dst // NPC
    srow = row_of[src]
    dslot = slot_of[dst]
    dblock = dslot // BLK
    dpos = dslot % BLK
    ishi = (srow >= cfg.HALF).astype(np.int64)

    # per (core, block, half) edge segment sizes -> shared chunk counts
    cnt = np.zeros((C, NB, 2), np.int64)
    np.add.at(cnt, (ecore, dblock, ishi), 1)
    cpb_lo = np.maximum(1, -(-cnt[:, :, 0].max(axis=0) // P))
    cpb_hi = -(-cnt[:, :, 1].max(axis=0) // P)
    if cfg.R > cfg.HALF:
        cpb_hi = np.maximum(cpb_hi, 1)

    lo_base = np.concatenate([[0], np.cumsum(cpb_lo)])  # chunk-id bases per block
    hi_base = np.concatenate([[0], np.cumsum(cpb_hi)])
    TC_lo, TC_hi = int(lo_base[-1]), int(hi_base[-1])
    TC = TC_lo + TC_hi

    # --- per-core padded edge arrays: gather idx (int16, wrapped) + one-hots ---
    idx_w_all, oh_all = [], []
    for c in range(C):
        m = ecore == c
        es, eb, ep, eh = srow[m], dblock[m], dpos[m], ishi[m]
        einv = inv[dst[m]]
        order = np.lexsort((es, eh, eb))
        es, eb, ep, eh, einv = es[order], eb[order], ep[order], eh[order], einv[order]

        idx_flat = np.zeros(TC * P, np.int64)
        oh = np.zeros((P, TC * BLK), np.float32)
        seg_cnt = np.zeros((NB, 2), np.int64)
        np.add.at(seg_cnt, (eb, eh), 1)
        pos = 0
        for b in range(NB):
            for h in (0, 1):
                k = int(seg_cnt[b, h])
                if k:
                    sl = slice(pos, pos + k)
                    base = (lo_base[b] if h == 0 else TC_lo + hi_base[b]) * P
                    gpos = base + np.arange(k)
                    idx_flat[gpos] = es[sl] - (cfg.HALF if h else 0)
                    cid = gpos // P
                    lane = gpos % P
                    oh[lane, cid * BLK + ep[sl]] = einv[sl]
                    pos += k
        assert pos == int(m.sum())
        assert idx_flat.max() <= 32767 and idx_flat.min() >= 0
        idx_w = np.zeros((P, TC * P // 16), np.int16)
        # wrapped in 16 partitions AND replicated across the 8 Q7 cores
        idx_w[:, :] = np.tile(idx_flat.reshape(-1, 16).T.astype(np.int16), (8, 1))
        idx_w_all.append(idx_w)
        oh_all.append(oh.astype(ml_dtypes.bfloat16))

    plan = Plan()
    plan.cpb_lo, plan.cpb_hi = cpb_lo, cpb_hi
    plan.lo_base, plan.hi_base = lo_base, hi_base
    plan.TC_lo, plan.TC_hi, plan.TC = TC_lo, TC_hi, TC
    plan.idx_w = idx_w_all
    plan.oh = oh_all
    plan.slot_of = slot_of
    plan.row_of = row_of
    return plan


def prepare_inputs(cfg, plan, inputs):
    """Build per-core in_map dicts."""
    x = np.asarray(inputs["x"], np.float32)
    C, NPC, SLOTS = cfg.C, cfg.NPC, cfg.SLOTS
    F = cfg.dims[0]
    Fp = _pad128(F)

    in_maps = [dict() for _ in range(C)]
    for c in range(C):
        xs = np.zeros((SLOTS, Fp), np.float32)
        nodes = np.arange(c * NPC, (c + 1) * NPC)
        xs[plan.slot_of[nodes], :F] = x[nodes]
        in_maps[c]["xT"] = np.ascontiguousarray(xs.T).astype(ml_dtypes.bfloat16)
        in_maps[c]["oh"] = plan.oh[c]
        in_maps[c]["idx"] = plan.idx_w[c]

    for k in range(cfg.L):
        din, dout = cfg.dims[k], cfg.dims[k + 1]
        dinp = _pad128(din)
        Wl = np.asarray(inputs[f"Wl{k + 1}"], np.float32)
        Wr = np.asarray(inputs[f"Wr{k + 1}"], np.float32)
        b = np.asarray(inputs[f"b{k + 1}"], np.float32)
        if cfg.mode[k] == "T":
            Wlp = np.zeros((dinp, dout), np.float32)
            Wlp[:din] = Wl
        else:
            Wlp = Wl  # applied to agg (dim == din, already a 128-multiple)
        Wrp = np.zeros((dinp, dout), np.float32)
        Wrp[:din] = Wr

        def _ktiled(W):  # [K*128, dout] -> [128, K*dout] (K-tiles side by side)
            Kt = W.shape[0] // P
            return np.ascontiguousarray(
                W.reshape(Kt, P, dout).transpose(1, 0, 2).reshape(P, Kt * dout)
            )

        for c in range(C):
            in_maps[c][f"Wl{k + 1}"] = _ktiled(Wlp).astype(ml_dtypes.bfloat16)
            in_maps[c][f"Wr{k + 1}"] = _ktiled(Wrp).astype(ml_dtypes.bfloat16)
            in_maps[c][f"b{k + 1}"] = b[None, :].astype(ml_dtypes.bfloat16)
    return in_maps


# ---------------------------------------------------------------------------
# device program
# ---------------------------------------------------------------------------


def build(tc, outs, ins, cfg, plan, dbg=False):
    """Emit the whole program. `ins`/`outs` are dicts name -> AP."""
    nc = tc.nc
    C, NB, SLOTS, R, L = cfg.C, cfg.NB, cfg.SLOTS, cfg.R, cfg.L
    NPAIR = NB // 2
    TC_lo, TC_hi, TC = plan.TC_lo, plan.TC_hi, plan.TC
    GTC = cfg.GTC

    out_ext = outs["out"]

    # ---- internal DRAM ----
    h_dram = [None] * (L + 1)
    table = [None] * L
    bounce = [None] * L
    aggm_dram = {}
    t_addr = "Shared" if C > 4 else "Local"  # Shared collective out unsupported <5 cores
    for k in range(L):
        d = cfg.tdim[k]
        table[k] = nc.dram_tensor(f"table{k}", [R, d], BF16, kind="Internal", addr_space=t_addr)
        if cfg.mode[k] == "T":
            bounce[k] = nc.dram_tensor(f"bounce{k}", [SLOTS, d], BF16, kind="Internal")
        else:
            aggm_dram[k] = nc.dram_tensor(f"aggm{k}", [SLOTS, cfg.dims[k]], BF16, kind="Internal")
        if k < L - 1:
            h_dram[k + 1] = nc.dram_tensor(
                f"h{k + 1}", [SLOTS, cfg.dims[k + 1]], BF16, kind="Internal"
            )
    hrb1_dram = nc.dram_tensor("hrb1", [SLOTS, cfg.dims[1]], BF16, kind="Internal")

    ctx = ExitStack()
    const = ctx.enter_context(tc.tile_pool(name="const", bufs=1))
    wpool = ctx.enter_context(tc.tile_pool(name="w", bufs=4))
    hTp = ctx.enter_context(tc.tile_pool(name="hTp", bufs=4))
    glo_p = ctx.enter_context(tc.tile_pool(name="glo", bufs=3))
    ghi_p = ctx.enter_context(tc.tile_pool(name="ghi", bufs=2))
    wk = ctx.enter_context(tc.tile_pool(name="wk", bufs=4))
    xp = ctx.enter_context(tc.tile_pool(name="xp", bufs=3))
    ps_l = ctx.enter_context(tc.tile_pool(name="ps_l", bufs=4, space="PSUM"))
    ps_agg = ctx.enter_context(tc.tile_pool(name="ps_agg", bufs=2, space="PSUM"))

    # ---- persistent SBUF: one-hots, gather indices, ones row ----
    oh_sb = const.tile([P, TC * BLK], BF16, tag="oh")
    nc.sync.dma_start(oh_sb[:], ins["oh"])
    idx_sb = const.tile([P, TC * P // 16], I16, tag="idx")
    nc.sync.dma_start(idx_sb[:], ins["idx"])
    ones_sb = const.tile([1, P], BF16, tag="ones")
    nc.vector.memset(ones_sb[:], 1.0)

    def load_weights(k):
        """Wl/Wr as [128, K*dout] SBUF tiles (K-tiles side by side) + bias row."""
        din_l = cfg.dims[k] if cfg.mode[k] == "S" else _pad128(cfg.dims[k])
        dinp = _pad128(cfg.dims[k])
        dout = cfg.dims[k + 1]
        Kl, Kr = din_l // P, dinp // P
        wl = wpool.tile([P, Kl * dout], BF16, tag="w")
        nc.sync.dma_start(wl[:], ins[f"Wl{k + 1}"])
        wr = wpool.tile([P, Kr * dout], BF16, tag="w")
        nc.sync.dma_start(wr[:], ins[f"Wr{k + 1}"])
        bia = wpool.tile([1, dout], BF16, tag="bias")
        nc.sync.dma_start(bia[:], ins[f"b{k + 1}"])
        return wl, wr, bia, Kl, Kr

    # hT slabs of the current h (feature-major [128, SLOTS]); layer-1 input is
    # streamed from xT instead.
    hT_cur = None

    def make_hT(src_dram, d):
        slabs = []
        for ksl in range(d // P):
            sl = hTp.tile([P, SLOTS], BF16, tag="hT")
            nc.sync.dma_start_transpose(sl[:], src_dram[:, ksl * P : (ksl + 1) * P])
            slabs.append(sl)
        return slabs

    def emit_gather_scatter(k, pre_cb):
        """Lazily gather G tiles; per block-pair emit fused pre-matmuls (via
        pre_cb) followed by one-hot scatter matmuls, all accumulating into one
        [128, d] PSUM tile (two 64-row halves). Yields (pair, psum)."""
        d = cfg.tdim[k]
        gtiles = {}

        def get_gt(stream, t):
            key = (stream, t)
            if key not in gtiles:
                tc_s = TC_lo if stream == 0 else TC_hi
                base_col = 0 if stream == 0 else TC_lo * 8
                nch = min(GTC, tc_s - t * GTC)
                pool = glo_p if stream == 0 else ghi_p
                gt = pool.tile([P, GTC * d], BF16, tag="g")
                nidx = nch * P
                if stream == 0:
                    src = table[k][0 : cfg.HALF, :]
                else:
                    src = table[k][cfg.HALF : R, :]
                nc.gpsimd.dma_gather(
                    out_ap=gt[:, : nch * d].rearrange("p (c d) -> p c d", d=d),
                    in_ap=src,
                    idxs_ap=idx_sb[:, base_col + t * GTC * 8 : base_col + (t * GTC + nch) * 8],
                    num_idxs=nidx,
                    num_idxs_reg=nidx,
                    elem_size=d,
                )
                gtiles[key] = gt
            return gtiles[key]

        for pair in range(NPAIR):
            pagg = ps_agg.tile([P, d], F32, tag="agg")
            for half in (0, 1):
                b = pair * 2 + half
                hs = half * BLK
                n_pre = pre_cb(half, b, pagg)
                n_chunks = int(plan.cpb_lo[b] + plan.cpb_hi[b])
                tot = n_pre + n_chunks
                i = n_pre
                for stream, n_j, basearr in ((0, int(plan.cpb_lo[b]), plan.lo_base), (1, int(plan.cpb_hi[b]), plan.hi_base)):
                    for j in range(n_j):
                        cid = int(basearr[b]) + j
                        gt = get_gt(stream, cid // GTC)
                        s = cid % GTC
                        ohc = cid if stream == 0 else TC_lo + cid
                        nc.tensor.matmul(
                            pagg[hs : hs + BLK, :],
                            lhsT=oh_sb[:, ohc * BLK : (ohc + 1) * BLK],
                            rhs=gt[:, s * d : (s + 1) * d],
                            start=(i == 0),
                            stop=(i == tot - 1),
                            tile_position=(0, hs),
                        )
                        i += 1
            yield pair, pagg

    # ------------------------------------------------------------------
    for k in range(L):
        din, dout = cfg.dims[k], cfg.dims[k + 1]
        d = cfg.tdim[k]
        wl, wr, bia, Kl, Kr = load_weights(k)
        last = k == L - 1

        if cfg.mode[k] == "T":
            # ---- hl transform -> bounce (layer 1 also computes hrb -> DRAM) ----
            if k == 0:
                NCH = SLOTS // P
                GRP = cfg.GRP
                for g0 in range(0, NCH, GRP):
                    ncs = list(range(g0, min(NCH, g0 + GRP)))
                    pls = [ps_l.tile([P, dout], F32, tag="pl", name=f"pl{g0}_{i}") for i in range(len(ncs))]
                    prs = [ps_l.tile([P, dout], F32, tag="pl", name=f"pr{g0}_{i}") for i in range(len(ncs))]
                    for kt in range(Kl):
                        piece = xp.tile([P, GRP * P], BF16, tag="xp")
                        w = len(ncs) * P
                        nc.sync.dma_start(
                            piece[:, :w], ins["xT"][kt * P : (kt + 1) * P, g0 * P : g0 * P + w]
                        )
                        for i in range(len(ncs)):
                            nc.tensor.matmul(
                                pls[i][:],
                                lhsT=piece[:, i * P : (i + 1) * P],
                                rhs=wl[:, kt * dout : (kt + 1) * dout],
                                start=(kt == 0),
                                stop=(kt == Kl - 1),
                            )
                            nc.tensor.matmul(
                                prs[i][:],
                                lhsT=piece[:, i * P : (i + 1) * P],
                                rhs=wr[:, kt * dout : (kt + 1) * dout],
                                start=(kt == 0),
                                stop=False,
                            )
                    for i, nc_ in enumerate(ncs):
                        # bias into psum_r via ones-row outer product
                        nc.tensor.matmul(
                            prs[i][:], lhsT=ones_sb[:], rhs=bia[:], start=False, stop=True
                        )
                        hlt = wk.tile([P, dout], BF16, tag="hlt")
                        nc.vector.tensor_copy(out=hlt[:], in_=pls[i][:])
                        nc.sync.dma_start(bounce[k][nc_ * P : (nc_ + 1) * P, :], hlt[:])
                        hrt = wk.tile([P, dout], BF16, tag="hlt")
                        nc.vector.tensor_copy(out=hrt[:], in_=prs[i][:])
                        nc.sync.dma_start(hrb1_dram[nc_ * P : (nc_ + 1) * P, :], hrt[:])
            else:
                for nc_ in range(SLOTS // P):
                    pl = ps_l.tile([P, dout], F32, tag="pl")
                    for kt in range(Kl):
                        nc.tensor.matmul(
                            pl[:],
                            lhsT=hT_cur[kt][:, nc_ * P : (nc_ + 1) * P],
                            rhs=wl[:, kt * dout : (kt + 1) * dout],
                            start=(kt == 0),
                            stop=(kt == Kl - 1),
                        )
                    hlt = wk.tile([P, dout], BF16, tag="hlt")
                    nc.vector.tensor_copy(out=hlt[:], in_=pl[:])
                    nc.sync.dma_start(bounce[k][nc_ * P : (nc_ + 1) * P, :], hlt[:])
            ag_in = bounce[k]
        else:
            ag_in = h_dram[k]

        nc.gpsimd.collective_compute(
            "AllGather",
            mybir.AluOpType.bypass,
            replica_groups=[list(range(C))],
            ins=[ag_in[:].opt()],
            outs=[table[k][:].opt()],
        )
        if dbg and k == 0:
            nc.sync.dma_start(outs["dbg_bounce"][:], ag_in[:])
            if cfg.mode[0] == "T":
                nc.sync.dma_start(outs["dbg_hrb"][:], hrb1_dram[:])

        # ---- scatter (+fused hr/bias for mode T, k>0) ----
        if cfg.mode[k] == "T":
            if k == 0:

                def pre_cb(half, b, pagg):
                    return 0

            else:
                hT_in = hT_cur

                def pre_cb(half, b, pagg, _wr=wr, _Kr=Kr, _bia=bia, _dout=dout, _hT=hT_in):
                    hs = half * BLK
                    for kt in range(_Kr):
                        nc.tensor.matmul(
                            pagg[hs : hs + BLK, :],
                            lhsT=_hT[kt][:, b * BLK : (b + 1) * BLK],
                            rhs=_wr[:, kt * _dout : (kt + 1) * _dout],
                            start=(kt == 0),
                            stop=False,
                            tile_position=(0, hs),
                        )
                    nc.tensor.matmul(
                        pagg[hs : hs + BLK, :],
                        lhsT=ones_sb[:, :BLK],
                        rhs=_bia[:],
                        start=False,
                        stop=False,
                        tile_position=(0, hs),
                    )
                    return _Kr + 1

            for pair, pagg in emit_gather_scatter(k, pre_cb):
                if k == 0:
                    hrt = wk.tile([P, dout], BF16, tag="hrt")
                    nc.sync.dma_start(hrt[:], hrb1_dram[pair * P : (pair + 1) * P, :])
                    t0 = wk.tile([P, dout], F32, tag="t0")
                    nc.vector.tensor_tensor(
                        out=t0[:], in0=pagg[:], in1=hrt[:], op=mybir.AluOpType.add
                    )
                    pre = t0
                else:
                    pre = pagg
                if cfg.norm[k]:
                    t1 = wk.tile([P, dout], F32, tag="t0")
                    nc.vector.tensor_copy(out=t1[:], in_=pre[:])
                    sq = wk.tile([P, dout], F32, tag="t0")
                    nc.vector.tensor_tensor(
                        out=sq[:], in0=t1[:], in1=t1[:], op=mybir.AluOpType.mult
                    )
                    ss = wk.tile([P, 1], F32, tag="ss")
                    nc.vector.tensor_reduce(
                        out=ss[:], in_=sq[:], axis=mybir.AxisListType.X, op=mybir.AluOpType.add
                    )
                    nrm = wk.tile([P, 1], F32, tag="ss")
                    nc.scalar.activation(nrm[:], ss[:], mybir.ActivationFunctionType.Sqrt)
                    nrm2 = wk.tile([P, 1], F32, tag="ss")
                    nc.vector.tensor_scalar_max(out=nrm2[:], in0=nrm[:], scalar1=1e-12)
                    rinv = wk.tile([P, 1], F32, tag="ss")
                    nc.vector.reciprocal(out=rinv[:], in_=nrm2[:])
                    tn = wk.tile([P, dout], F32, tag="t0")
                    nc.vector.tensor_scalar_mul(out=tn[:], in0=t1[:], scalar1=rinv[:])
                    pre = tn
                if last:
                    of = wk.tile([P, dout], F32, tag="of")
                    nc.vector.tensor_scalar_max(out=of[:], in0=pre[:], scalar1=0.0)
                    nc.sync.dma_start(out_ext[pair * P : (pair + 1) * P, :], of[:])
                else:
                    ht = wk.tile([P, dout], BF16, tag="ht")
                    nc.vector.tensor_scalar_max(out=ht[:], in0=pre[:], scalar1=0.0)
                    nc.sync.dma_start(h_dram[k + 1][pair * P : (pair + 1) * P, :], ht[:])
        else:

            def pre_cb(half, b, pagg):
                return 0

            for pair, pagg in emit_gather_scatter(k, pre_cb):
                at = wk.tile([P, d], BF16, tag="ht")
                nc.vector.tensor_copy(out=at[:], in_=pagg[:])
                nc.sync.dma_start(aggm_dram[k][pair * P : (pair + 1) * P, :], at[:])
            aggT = make_hT(aggm_dram[k], d)
            # out = aggT@Wl + hT@Wr + bias
            for nc_ in range(SLOTS // P):
                po = ps_l.tile([P, dout], F32, tag="pl")
                for kt in range(Kl):
                    nc.tensor.matmul(
                        po[:],
                        lhsT=aggT[kt][:, nc_ * P : (nc_ + 1) * P],
                        rhs=wl[:, kt * dout : (kt + 1) * dout],
                        start=(kt == 0),
                        stop=False,
                    )
                for kt in range(Kr):
                    nc.tensor.matmul(
                        po[:],
                        lhsT=hT_cur[kt][:, nc_ * P : (nc_ + 1) * P],
                        rhs=wr[:, kt * dout : (kt + 1) * dout],
                        start=False,
                        stop=False,
                    )
                nc.tensor.matmul(po[:], lhsT=ones_sb[:], rhs=bia[:], start=False, stop=True)
                if last:
                    of = wk.tile([P, dout], F32, tag="of")
                    nc.vector.tensor_scalar_max(out=of[:], in0=po[:], scalar1=0.0)
                    nc.sync.dma_start(out_ext[nc_ * P : (nc_ + 1) * P, :], of[:])
                else:
                    ht = wk.tile([P, dout], BF16, tag="ht")
                    nc.vector.tensor_scalar_max(out=ht[:], in0=po[:], scalar1=0.0)
                    nc.sync.dma_start(h_dram[k + 1][nc_ * P : (nc_ + 1) * P, :], ht[:])

        if not last:
            hT_cur = make_hT(h_dram[k + 1], dout)
            if dbg:
                nc.sync.dma_start(outs[f"dbg{k + 1}"][:], h_dram[k + 1][:])

    ctx.close()


# ---------------------------------------------------------------------------
# entry point
# ---------------------------------------------------------------------------


def _run(cfg, inputs, trace=False, dbg=False):
    plan = make_plan(cfg, np.asarray(inputs["edge_index"]))
    in_maps = prepare_inputs(cfg, plan, inputs)

    nc = bacc.Bacc("TRN2", target_bir_lowering=False, debug=False, num_devices=cfg.C)
    ins = {}
    for name, arr in in_maps[0].items():
        ins[name] = nc.dram_tensor(
            name, list(arr.shape), mybir.dt.from_np(arr.dtype), kind="ExternalInput"
        ).ap()
    outs = {"out": nc.dram_tensor("out", [cfg.SLOTS, cfg.dims[-1]], F32, kind="ExternalOutput").ap()}
    if dbg:
        for k in range(1, cfg.L):
            outs[f"dbg{k}"] = nc.dram_tensor(
                f"dbg{k}", [cfg.SLOTS, cfg.dims[k]], BF16, kind="ExternalOutput"
            ).ap()
        d0 = cfg.tdim[0]
        outs["dbg_bounce"] = nc.dram_tensor(
            "dbg_bounce", [cfg.SLOTS, d0], BF16, kind="ExternalOutput"
        ).ap()
        outs["dbg_hrb"] = nc.dram_tensor(
            "dbg_hrb", [cfg.SLOTS, cfg.dims[1]], BF16, kind="ExternalOutput"
        ).ap()
    with tile.TileContext(nc, num_cores=cfg.C) as tc:
        build(tc, outs, ins, cfg, plan, dbg=dbg)
    nc.finalize()

    try:
        res = run_bass_kernel_spmd(nc, in_maps, core_ids=list(range(cfg.C)), trace=trace)
    except ModuleNotFoundError:
        # no axon NTFF profiling hook in this container
        res = run_bass_kernel_spmd(nc, in_maps, core_ids=list(range(cfg.C)), trace=False)
    # reassemble: out[node] = shard[core][slot]
    full = np.empty((cfg.N, cfg.dims[-1]), np.float32)
    for c in range(cfg.C):
        o = res.results[c]["out"]
        nodes = np.arange(c * cfg.NPC, (c + 1) * cfg.NPC)
        full[nodes] = o[plan.slot_of[nodes]]
    return full, res


def kernel(**inputs) -> np.ndarray:
    cfg = Cfg(N=50000, E=400000, C=8, dims=REAL_DIMS, norm=REAL_NORM)
    full, _ = _run(cfg, inputs)
    return full
